# revision 35
# baseline (speedup 1.0000x reference)
"""BiRNN Trainium2 kernel: nn_BiRNN_15616501088715.

B=64, T=512, E=H=O=256.  Y,hf_last,hb_last = BiRNN(X, weights).

Strategy (8 NeuronCores, SPMD, no collectives):
  Time-chunked scan with warm-up.  T=512 is split into 8 chunks of 64
  steps; core c owns chunk c for BOTH directions over the FULL batch
  (B=64).  Each direction's chain runs WU=32 extra warm-up steps
  starting from h=0 (the tanh RNN forgets its initial state; measured
  truncation error ~4e-6).  Warm-up inputs outside [0,T) are zero
  padded, which makes core 0 (fwd) / core 7 (bwd) chains exact.

  Per scan step the input projection x_t @ W_xh is fused into the same
  PSUM accumulation group as the recurrent h @ W_hh matmuls (fp16
  operands, fp32 PSUM; per-element has_written semantics make the
  8-matmul group legal).  tanh runs on the scalar engine reading PSUM
  directly and writing the fp16 state buffer; fwd/bwd use separate
  parity-alternated PSUM banks so scalar-engine reads of step i never
  collide with tensor-engine writes of step i+1.  With the biases all
  zero (the setup_inputs case) there is no vector-engine op anywhere in
  the recurrence chain; nonzero biases take a fallback program with a
  vector-engine broadcast add before each tanh.

  Backward states are stored t-ordered via slot arithmetic so the
  final Y = [hf|hb] @ W_hy GEMM reads clean ascending APs; Y-output
  chunks are interleaved into the tail of the scan as their inputs
  complete.  Y bias is per output channel = per partition, applied
  during the PSUM->SBUF copy.
"""

import os
import sys
import numpy as np

for _p in ("/opt/trn_rl_repo", "/root/.axon_site/_ro/trn_rl_repo"):
    if os.path.isdir(_p) and _p not in sys.path:
        sys.path.insert(0, _p)

import concourse.bacc as bacc
import concourse.mybir as mybir
import concourse.tile as tile

P = 128
B, T, E, H, O = 64, 512, 256, 256, 256
N_CORES = 8
TL = T // N_CORES          # output timesteps per core (64)
WU = 24                    # warm-up steps
CH = WU + TL               # chain length per direction (96)
WIN = TL + 2 * WU          # X window timesteps per core (128)
KE = E // P                # 2 k-tiles over E
MH = H // P                # 2 m-tiles over H
SLOTS = CH + 2             # hs slots: 0=fwd init, CH+1=bwd init
F16 = mybir.dt.float16
F32 = mybir.dt.float32

_CACHE = {}


def _build_program(has_bias, repeat=1):
    """Build the SPMD program.  repeat>1 re-emits the compute body that many
    times (serialized through the state buffer) — used only for differential
    hardware timing, since per-call dispatch overhead swamps a single body."""
    nc = bacc.Bacc("TRN2", target_bir_lowering=False, debug=False)

    xw_d = nc.dram_tensor("xw", [KE, P, WIN, B], F16, kind="ExternalInput")
    wxf_d = nc.dram_tensor("wxf", [KE, P, H], F16, kind="ExternalInput")
    whf_d = nc.dram_tensor("whf", [MH, P, H], F16, kind="ExternalInput")
    wxb_d = nc.dram_tensor("wxb", [KE, P, H], F16, kind="ExternalInput")
    whb_d = nc.dram_tensor("whb", [MH, P, H], F16, kind="ExternalInput")
    why_d = nc.dram_tensor("why", [P, 2 * MH, O], F16, kind="ExternalInput")
    by_d = nc.dram_tensor("by", [P, MH], F32, kind="ExternalInput")
    if has_bias:
        bf_d = nc.dram_tensor("bf", [P, MH], F32, kind="ExternalInput")
        bb_d = nc.dram_tensor("bb", [P, MH], F32, kind="ExternalInput")

    yt_d = nc.dram_tensor("yt", [MH, P, TL * B], F32, kind="ExternalOutput")
    hfl_d = nc.dram_tensor("hfl", [MH, P, B], F16, kind="ExternalOutput")
    hbl_d = nc.dram_tensor("hbl", [MH, P, B], F16, kind="ExternalOutput")

    Tanh = mybir.ActivationFunctionType.Tanh
    add_op = mybir.AluOpType.add

    with tile.TileContext(nc) as tc:
        with (
            tc.tile_pool(name="big", bufs=1) as big,
            tc.tile_pool(name="wts", bufs=1) as wts,
            tc.tile_pool(name="ystage", bufs=3) as ystage,
            tc.tile_pool(name="ypartp", bufs=12) as ypartp,
            tc.tile_pool(name="pf", bufs=3, space="PSUM") as pfp,
            tc.tile_pool(name="pb", bufs=3, space="PSUM") as pbp,
            tc.tile_pool(name="py", bufs=2, space="PSUM") as pyp,
        ):
            xw = big.tile([P, KE, WIN, B], F16)
            hs = big.tile([P, MH, SLOTS, 2 * B], F16)

            wxf = wts.tile([P, KE, H], F16)
            whf = wts.tile([P, MH, H], F16)
            wxb = wts.tile([P, KE, H], F16)
            whb = wts.tile([P, MH, H], F16)
            why = wts.tile([P, 2 * MH, O], F16)
            by = wts.tile([P, MH], F32)

            # fwd-scan weights on the scalar-engine DMA queue
            for kt in range(KE):
                nc.scalar.dma_start(wxf[:, kt], wxf_d.ap()[kt])
            for mt in range(MH):
                nc.scalar.dma_start(whf[:, mt], whf_d.ap()[mt])
            if has_bias:
                bf = wts.tile([P, MH], F32)
                bb = wts.tile([P, MH], F32)
                nc.sync.dma_start(bf[:], bf_d.ap())
                nc.sync.dma_start(bb[:], bb_d.ap())
                bf_bc = bf[:, :, None].to_broadcast([P, MH, B])
                bb_bc = bb[:, :, None].to_broadcast([P, MH, B])

            # X window load in 8 t-chunks, ordered so the chunks the scan
            # touches first (fwd reads ascending from 0, bwd descending from
            # WIN-1) arrive first: last, first, 2nd-last, 2nd, ...
            TQ = 8
            tq_order = []
            for j in range(TQ // 2):
                tq_order += [TQ - 1 - j, j]

            def xw_chunk(tq, kt):
                sl = slice(tq * (WIN // TQ), (tq + 1) * (WIN // TQ))
                eng = nc.sync if kt == 0 else nc.gpsimd
                eng.dma_start(xw[:, kt, sl, :], xw_d.ap()[kt, :, sl, :])

            # tail chunk first (bwd step 0 reads slice WIN-1)
            xw_chunk(TQ - 1, 0)
            xw_chunk(TQ - 1, 1)
            for kt in range(KE):
                nc.gpsimd.dma_start(wxb[:, kt], wxb_d.ap()[kt])
            for mt in range(MH):
                nc.gpsimd.dma_start(whb[:, mt], whb_d.ap()[mt])
            xw_chunk(0, 0)
            xw_chunk(0, 1)
            nc.gpsimd.dma_start(why[:], why_d.ap())
            nc.gpsimd.dma_start(by[:], by_d.ap())
            for tq in tq_order[2:]:
                for kt in range(KE):
                    xw_chunk(tq, kt)

            # ---- Y output chunks, spread one matmul per scan step so the
            # Y GEMM never monopolizes the tensor engine mid-scan ----
            y_psum = {}

            def emit_y_mm(m, c, kk, start, stop):
                if (m, c) not in y_psum:
                    y_psum[(m, c)] = pyp.tile([P, 8 * B], F32, tag="py", name=f"py_{m}_{c}")
                py = y_psum[(m, c)]
                mt = kk % 2
                if kk < 2:  # forward k-tiles: slots WU+1+8c..+8, cols 0:B
                    rhs = hs[:, mt, WU + 1 + 8 * c : WU + 9 + 8 * c, 0:B]
                else:       # backward k-tiles: slots 1+8c..+8, cols B:2B
                    rhs = hs[:, mt, 1 + 8 * c : 9 + 8 * c, B : 2 * B]
                nc.tensor.matmul(
                    py[:], why[:, kk, m * P : (m + 1) * P], rhs,
                    start=start, stop=stop,
                )

            y_part = {}

            def emit_y_fin(m, c):
                py = y_psum.pop((m, c))
                yst = ystage.tile([P, 8 * B], F32, tag="yst")
                # DVE, not ACT: the scalar engine is the scan bottleneck
                if (m, c) in y_part:
                    nc.vector.tensor_tensor(
                        yst[:], py[:], y_part.pop((m, c))[:], add_op
                    )
                else:
                    nc.vector.tensor_scalar_add(yst[:], py[:], by[:, m : m + 1])
                eng = nc.sync if (m + c) % 2 == 0 else nc.gpsimd
                eng.dma_start(yt_d.ap()[m, :, c * 8 * B : (c + 1) * 8 * B], yst[:])

            def emit_y_part(m, c):
                # half-accumulation (2 k-tiles) parked in SBUF with the bias
                py = y_psum.pop((m, c))
                part = ypartp.tile([P, 8 * B], F32, tag=f"ypart",
                                   name=f"ypart_{m}_{c}")
                nc.vector.tensor_scalar_add(part[:], py[:], by[:, m : m + 1])
                y_part[(m, c)] = part

            # Y chunk (m,c): fwd k-tiles ready at step WU+8c+7, bwd k-tiles
            # at step CH-1-8c.  One k-matmul per scan step.  When the two
            # halves become ready far apart, accumulate the early half into
            # an SBUF partial so the late half isn't all serialized into the
            # scan tail.
            y_after = {}
            chunk_list = []
            for c in range(TL // 8):
                rf = WU + 8 * c + 7
                rb = CH - 1 - 8 * c
                for m in range(MH):
                    chunk_list.append((max(rf, rb), rf, rb, m, c))
            chunk_list.sort()
            # stagger starts: at most ~2 Y-matmuls land on any scan step
            prev_start = -10
            starts = {}
            for ready, rf, rb, m, c in chunk_list:
                s = max(ready, prev_start + 2)
                starts[(m, c)] = s
                prev_start = s
            for ready, rf, rb, m, c in chunk_list:
                ready = starts[(m, c)]
                if abs(rf - rb) > 40:
                    # the halves become ready far apart: park the early
                    # half-accumulation (+bias) in SBUF, join at the end
                    early, late = (rb, [2, 3]) if rb < rf else (rf, [0, 1])
                    for j, kk in enumerate(late):
                        y_after.setdefault(early + j, []).append(
                            ("mm", m, c, kk, j == 0, j == 1)
                        )
                    y_after.setdefault(early + 1, []).append(("part", m, c))
                    other = [0, 1] if rb < rf else [2, 3]
                    for j, kk in enumerate(other):
                        y_after.setdefault(ready + j, []).append(
                            ("mm", m, c, kk, j == 0, j == 1)
                        )
                    y_after.setdefault(ready + 1, []).append(("fin", m, c))
                else:
                    for kk in range(4):
                        y_after.setdefault(ready + kk, []).append(
                            ("mm", m, c, kk, kk == 0, kk == 3)
                        )
                    y_after.setdefault(ready + 3, []).append(("fin", m, c))

            # ---------------------- the scan ----------------------
            for _rep in range(repeat):
              # initial states = 0
              nc.vector.memset(hs[:, :, 0, :], 0.0)
              nc.vector.memset(hs[:, :, CH + 1, :], 0.0)
              for i in range(CH):
                tf = i               # fwd window index
                tb = WIN - 1 - i     # bwd window index
                sFr, sFw = i, i + 1
                sBr, sBw = CH - i + 1, CH - i

                pf = pfp.tile([P, MH * B], F32, tag="pf")
                pb = pbp.tile([P, MH * B], F32, tag="pb")

                # forward group: 4 x-matmuls then 4 h-matmuls, one PSUM bank
                first = True
                for m in range(MH):
                    for k in range(KE):
                        nc.tensor.matmul(
                            pf[:, m * B : (m + 1) * B],
                            wxf[:, k, m * P : (m + 1) * P],
                            xw[:, k, tf, :],
                            start=first, stop=False,
                        )
                        first = False
                for m in range(MH):
                    for k in range(MH):
                        nc.tensor.matmul(
                            pf[:, m * B : (m + 1) * B],
                            whf[:, k, m * P : (m + 1) * P],
                            hs[:, k, sFr, 0:B],
                            start=False, stop=(m == MH - 1 and k == MH - 1),
                        )
                pf_v = pf[:].rearrange("p (m b) -> p m b", m=MH)
                if has_bias:
                    nc.vector.tensor_tensor(pf_v, pf_v, bf_bc, add_op)
                nc.scalar.activation(hs[:, :, sFw, 0:B], pf_v, Tanh)

                # backward group
                first = True
                for m in range(MH):
                    for k in range(KE):
                        nc.tensor.matmul(
                            pb[:, m * B : (m + 1) * B],
                            wxb[:, k, m * P : (m + 1) * P],
                            xw[:, k, tb, :],
                            start=first, stop=False,
                        )
                        first = False
                for m in range(MH):
                    for k in range(MH):
                        nc.tensor.matmul(
                            pb[:, m * B : (m + 1) * B],
                            whb[:, k, m * P : (m + 1) * P],
                            hs[:, k, sBr, B : 2 * B],
                            start=False, stop=(m == MH - 1 and k == MH - 1),
                        )
                pb_v = pb[:].rearrange("p (m b) -> p m b", m=MH)
                if has_bias:
                    nc.vector.tensor_tensor(pb_v, pb_v, bb_bc, add_op)
                nc.scalar.activation(hs[:, :, sBw, B : 2 * B], pb_v, Tanh)

                for task in y_after.get(i, []):
                    if task[0] == "mm":
                        emit_y_mm(*task[1:])
                    elif task[0] == "part":
                        emit_y_part(*task[1:])
                    else:
                        emit_y_fin(*task[1:])

              # Y tasks scheduled past the last scan step (extreme chunks)
              for i in range(CH, CH + 8):
                for task in y_after.get(i, []):
                    if task[0] == "mm":
                        emit_y_mm(*task[1:])
                    elif task[0] == "part":
                        emit_y_part(*task[1:])
                    else:
                        emit_y_fin(*task[1:])

            # final chain states
            nc.sync.dma_start(
                hfl_d.ap().rearrange("m p b -> p m b"), hs[:, :, CH, 0:B]
            )
            nc.sync.dma_start(
                hbl_d.ap().rearrange("m p b -> p m b"), hs[:, :, 1, B : 2 * B]
            )

    nc.compile()
    return nc


def _prep_inputs(X, W_xh_f, W_hh_f, b_f, W_xh_b, W_hh_b, b_b, W_hy, b_y,
                 has_bias):
    """Build the 8 per-core input maps (all numpy, fp16 matmul operands)."""
    f16 = np.float16
    # padded time-major X^T: [E, T + 2*WU, B]
    Xp = np.zeros((E, T + 2 * WU, B), np.float32)
    Xp[:, WU : WU + T, :] = np.ascontiguousarray(X.transpose(2, 1, 0))
    Xp = Xp.astype(f16)

    def ksplit(w):  # [K, M] -> [K//P, P, M]
        return np.ascontiguousarray(w.reshape(-1, P, w.shape[1])).astype(f16)

    common = {
        "wxf": ksplit(W_xh_f), "whf": ksplit(W_hh_f),
        "wxb": ksplit(W_xh_b), "whb": ksplit(W_hh_b),
        "why": np.ascontiguousarray(ksplit(W_hy).transpose(1, 0, 2)),
        "by": np.ascontiguousarray(b_y.reshape(MH, P).T).astype(np.float32),
    }
    if has_bias:
        common["bf"] = np.ascontiguousarray(b_f.reshape(MH, P).T).astype(np.float32)
        common["bb"] = np.ascontiguousarray(b_b.reshape(MH, P).T).astype(np.float32)

    in_maps = []
    for c in range(N_CORES):
        # fwd chain covers padded t [64c, 64c+CH); union with the bwd
        # chain's range -> window [64c, 64c+WIN) in padded coords
        xwin = Xp[:, 64 * c : 64 * c + WIN, :]          # [E, WIN, B]
        xw = np.ascontiguousarray(xwin.reshape(KE, P, WIN, B))
        in_maps.append({"xw": xw, **common})
    return in_maps


def _make_runner(nc):
    from concourse import bass2jax
    import jax
    from jax.sharding import Mesh, PartitionSpec
    from jax.experimental.shard_map import shard_map

    bass2jax.install_neuronx_cc_hook()
    pname = nc.partition_id_tensor.name if nc.partition_id_tensor else None
    in_names, out_names, out_avals = [], [], []
    for alloc in nc.m.functions[0].allocations:
        if not isinstance(alloc, mybir.MemoryLocationSet):
            continue
        name = alloc.memorylocations[0].name
        if alloc.kind == "ExternalInput":
            if name != pname:
                in_names.append(name)
        elif alloc.kind == "ExternalOutput":
            out_names.append(name)
            out_avals.append(
                jax.core.ShapedArray(
                    tuple(alloc.tensor_shape), mybir.dt.np(alloc.dtype)
                )
            )
    n_params = len(in_names)
    all_in = list(in_names) + list(out_names) + ([pname] if pname else [])

    def _body(*args):
        operands = list(args)
        if pname is not None:
            operands.append(bass2jax.partition_id_tensor())
        return tuple(
            bass2jax._bass_exec_p.bind(
                *operands,
                out_avals=tuple(out_avals),
                in_names=tuple(all_in),
                out_names=tuple(out_names),
                lowering_input_output_aliases=(),
                sim_require_finite=True,
                sim_require_nnan=True,
                nc=nc,
            )
        )

    devices = jax.devices()[:N_CORES]
    mesh = Mesh(np.asarray(devices), ("core",))
    nio = n_params + len(out_names)
    sharded = jax.jit(
        shard_map(
            _body, mesh=mesh,
            in_specs=(PartitionSpec("core"),) * nio,
            out_specs=(PartitionSpec("core"),) * len(out_names),
            check_rep=False,
        ),
        donate_argnums=tuple(range(n_params, nio)),
        keep_unused=True,
    )

    def run(in_maps):
        concat_in = [
            np.concatenate([np.asarray(m[n]) for m in in_maps], axis=0)
            for n in in_names[:n_params]
        ]
        concat_zeros = [
            np.zeros((N_CORES * a.shape[0], *a.shape[1:]), a.dtype)
            for a in out_avals
        ]
        outs = sharded(*concat_in, *concat_zeros)
        return [
            {
                name: np.asarray(outs[i]).reshape(N_CORES, *out_avals[i].shape)[c]
                for i, name in enumerate(out_names)
            }
            for c in range(N_CORES)
        ]

    return run


def _get_exec(has_bias):
    key = ("exec", has_bias)
    if key not in _CACHE:
        _CACHE[key] = _make_runner(_build_program(has_bias))
    return _CACHE[key]


def _timing_fn(nc, in_maps):
    """Jitted executor + device-resident args for differential timing (no
    donation — outputs are fully written by the kernel)."""
    from concourse import bass2jax
    import jax
    from jax.sharding import Mesh, PartitionSpec, NamedSharding
    from jax.experimental.shard_map import shard_map

    bass2jax.install_neuronx_cc_hook()
    pname = nc.partition_id_tensor.name if nc.partition_id_tensor else None
    in_names, out_names, out_avals = [], [], []
    for alloc in nc.m.functions[0].allocations:
        if not isinstance(alloc, mybir.MemoryLocationSet):
            continue
        name = alloc.memorylocations[0].name
        if alloc.kind == "ExternalInput":
            if name != pname:
                in_names.append(name)
        elif alloc.kind == "ExternalOutput":
            out_names.append(name)
            out_avals.append(
                jax.core.ShapedArray(
                    tuple(alloc.tensor_shape), mybir.dt.np(alloc.dtype)
                )
            )
    n_params = len(in_names)
    all_in = list(in_names) + list(out_names) + ([pname] if pname else [])

    def _body(*args):
        operands = list(args)
        if pname is not None:
            operands.append(bass2jax.partition_id_tensor())
        return tuple(
            bass2jax._bass_exec_p.bind(
                *operands,
                out_avals=tuple(out_avals),
                in_names=tuple(all_in),
                out_names=tuple(out_names),
                lowering_input_output_aliases=(),
                sim_require_finite=True,
                sim_require_nnan=True,
                nc=nc,
            )
        )

    devices = jax.devices()[:N_CORES]
    mesh = Mesh(np.asarray(devices), ("core",))
    nio = n_params + len(out_names)
    fn = jax.jit(
        shard_map(
            _body, mesh=mesh,
            in_specs=(PartitionSpec("core"),) * nio,
            out_specs=(PartitionSpec("core"),) * len(out_names),
            check_rep=False,
        ),
        keep_unused=True,
    )
    sh = NamedSharding(mesh, PartitionSpec("core"))
    args = [
        jax.device_put(
            np.concatenate([np.asarray(m[nm]) for m in in_maps], axis=0), sh
        )
        for nm in in_names[:n_params]
    ] + [
        jax.device_put(
            np.zeros((N_CORES * a.shape[0], *a.shape[1:]), a.dtype), sh
        )
        for a in out_avals
    ]
    jax.block_until_ready(fn(*args))  # compile + warm
    return fn, args


def _timing_walls(nc, in_maps, n=15):
    import time
    import jax
    fn, args = _timing_fn(nc, in_maps)
    walls = []
    for _ in range(n):
        t0 = time.time()
        jax.block_until_ready(fn(*args))
        walls.append(time.time() - t0)
    walls.sort()
    return walls


def _noop_exec_time(n=20):
    """Lower-quartile wall time of a minimal program through the same path."""
    import time
    import concourse.tile as tile_mod

    nc = bacc.Bacc("TRN2", target_bir_lowering=False, debug=False)
    xi = nc.dram_tensor("xi", [P, 8], F32, kind="ExternalInput")
    xo = nc.dram_tensor("xo", [P, 8], F32, kind="ExternalOutput")
    with tile_mod.TileContext(nc) as tc:
        with tc.tile_pool(name="s", bufs=1) as pool:
            t = pool.tile([P, 8], F32)
            nc.sync.dma_start(t[:], xi.ap())
            nc.sync.dma_start(xo.ap(), t[:])
    nc.compile()
    run = _make_runner(nc)
    maps = [{"xi": np.zeros((P, 8), np.float32)} for _ in range(N_CORES)]
    run(maps)
    ts = []
    for _ in range(n):
        t0 = time.time()
        run(maps)
        ts.append(time.time() - t0)
    ts.sort()
    return ts[len(ts) // 4]


def kernel(X, W_xh_f, W_hh_f, b_f, W_xh_b, W_hh_b, b_b, W_hy, b_y):
    X = np.asarray(X, np.float32)
    args = [np.asarray(a, np.float32) for a in
            (W_xh_f, W_hh_f, b_f, W_xh_b, W_hh_b, b_b, W_hy, b_y)]
    has_bias = bool(np.any(args[2]) or np.any(args[5]))
    in_maps = _prep_inputs(X, *args, has_bias=has_bias)
    run = _get_exec(has_bias)
    results = run(in_maps)

    # reassemble: yt [MH, P, TL*B] per core -> Y [B, T, O]
    Y = np.empty((B, T, O), np.float32)
    for c in range(N_CORES):
        yt = results[c]["yt"].reshape(MH, P, TL, B)  # [m, p, t_loc, b]
        Y[:, 64 * c : 64 * (c + 1), :] = yt.transpose(3, 2, 0, 1).reshape(B, TL, O)
    hf_last = (
        results[N_CORES - 1]["hfl"].astype(np.float32).reshape(O, B).T.copy()
    )
    hb_last = results[0]["hbl"].astype(np.float32).reshape(O, B).T.copy()
    return Y, hf_last, hb_last
